# revision 1
# baseline (speedup 1.0000x reference)
"""TRN2 Bass kernel for nn_CNActorController (GRU comm-net actor).

Strategy (self-contained, shapes hardcoded):
- Pure data parallelism: batch dim sharded 8 ways (1024 samples / core).
- On-chip layout is feature-major ("transposed"): activations live as
  [feature partitions, row free-dim], rows ordered sample-major (b*16+a).
- The agent-mean communication step collapses (mask==const across agents):
    ci_n @ gru_k = (sum_a x)/16 @ gru_k + const
  so it is computed per-sample via a segmented row-sum + a small matmul,
  and all OU-noise terms + biases collapse into per-gate constants folded
  on the host.
- All matmuls run as float32r (TF32-like, 1 cyc/row on the PE at N>=256)
  with fp32 PSUM accumulation.
"""
import os
import numpy as np

import jax
try:
    _cache_dir = os.path.expanduser("~/.cache/jax_kernel_cache")
    os.makedirs(_cache_dir, exist_ok=True)
    jax.config.update("jax_compilation_cache_dir", _cache_dir)
    jax.config.update("jax_persistent_cache_min_entry_size_bytes", -1)
    jax.config.update("jax_persistent_cache_min_compile_time_secs", 2)
except Exception:
    pass

import concourse.bass as bass
import concourse.mybir as mybir
import concourse.tile as tile
from concourse import bacc
from concourse.bass_utils import run_bass_kernel_spmd
from concourse.masks import make_identity

# problem dims (hardcoded per spec)
B, A, D, H, C, NA = 8192, 16, 64, 256, 2, 5
NCORES = 8
B_LOC = B // NCORES          # 1024 samples per core
R_LOC = B_LOC * A            # 16384 rows per core
P = 128

SC_SAMPLES = 256             # samples per super-chunk
N_SC = B_LOC // SC_SAMPLES   # 4 super-chunks
CHUNK = 512                  # rows per chunk
CH_S = CHUNK // A            # 32 samples per chunk
N_CH = SC_SAMPLES * A // CHUNK  # 8 chunks per super-chunk
SC_ROWS = SC_SAMPLES * A     # 4096 rows per super-chunk

MT_G = 6                     # gate m-tiles (3H/128)
MT_H = 2                     # state m-tiles (H/128)
KT = 2                       # k-tiles for H contraction

f32 = mybir.dt.float32
f32r = mybir.dt.float32r
bf16 = mybir.dt.bfloat16
AF = mybir.ActivationFunctionType
ALU = mybir.AluOpType

_CACHE = {}
N_REPS = 1


def _build():
    """Build the per-core Bass module (same program for all 8 cores)."""
    nc = bacc.Bacc("TRN2", target_bir_lowering=False, debug=False,
                   num_devices=NCORES)

    # ---- DRAM I/O ----
    obs_d = nc.dram_tensor("obs", [D, R_LOC], f32, kind="ExternalInput")
    wenc_d = nc.dram_tensor("wenc", [D, H], f32, kind="ExternalInput")
    benc_d = nc.dram_tensor("benc", [H], f32, kind="ExternalInput")
    wenc2_d = nc.dram_tensor("wenc2", [H, H], f32, kind="ExternalInput")
    benc2_d = nc.dram_tensor("benc2", [H], f32, kind="ExternalInput")
    grk_d = nc.dram_tensor("grk", [C, H, 3 * H], f32, kind="ExternalInput")
    gks_d = nc.dram_tensor("gks", [C, H, 3 * H], f32, kind="ExternalInput")
    cmb_d = nc.dram_tensor("cmb", [C, 3 * H], f32, kind="ExternalInput")
    b1h_d = nc.dram_tensor("b1h", [C, H], f32, kind="ExternalInput")
    wdec_d = nc.dram_tensor("wdec", [H, H], f32, kind="ExternalInput")
    bdec_d = nc.dram_tensor("bdec", [H], f32, kind="ExternalInput")
    wout_d = nc.dram_tensor("wout", [H, NA], f32, kind="ExternalInput")
    bout_d = nc.dram_tensor("bout", [NA, 1], f32, kind="ExternalInput")
    e32_d = nc.dram_tensor("e32", [64, CHUNK], f32, kind="ExternalInput")
    out_d = nc.dram_tensor("out_t", [NA, R_LOC], f32, kind="ExternalOutput")

    with tile.TileContext(nc) as tc:
        with (
            tc.tile_pool(name="wp", bufs=1) as wp,          # weights, consts
            tc.tile_pool(name="st", bufs=2) as stg,         # dma staging
            tc.tile_pool(name="sc", bufs=2) as scp,         # per-superchunk h0
            tc.tile_pool(name="xp", bufs=2) as xp,          # x state buffer
            tc.tile_pool(name="sp", bufs=2) as sp,          # segsums + cm
            tc.tile_pool(name="tp", bufs=3) as tp,          # chunk transients
            tc.tile_pool(name="pg", bufs=4, space="PSUM") as pg,   # gate psums
            tc.tile_pool(name="pm", bufs=2, space="PSUM") as pm,   # misc psums
            tc.tile_pool(name="pt", bufs=2, space="PSUM") as pt,   # transpose
        ):
            # ---- load weights (f32 staged dma -> f32r rounded copies) ----
            def round_into(dst_ap, dram_ap, shape, name):
                raw = stg.tile(shape, f32, name=name, tag="raw")
                nc.sync.dma_start(raw[:], dram_ap)
                nc.vector.tensor_copy(dst_ap, raw[:])

            wenc = wp.tile([D, H], f32r, name="wenc")
            round_into(wenc[:], wenc_d[:], [D, H], "r_wenc")
            wenc2 = wp.tile([P, KT, H], f32r, name="wenc2")
            round_into(wenc2[:], wenc2_d.rearrange("(kt p) m -> p kt m", p=P),
                       [P, KT, H], "r_wenc2")
            grk = wp.tile([P, C, KT, 3 * H], bf16, name="grk")
            gks = wp.tile([P, C, KT, 3 * H], bf16, name="gks")
            for li in range(C):
                round_into(grk[:, li], grk_d[li].rearrange(
                    "(kt p) m -> p kt m", p=P), [P, KT, 3 * H], "r_grk")
                round_into(gks[:, li], gks_d[li].rearrange(
                    "(kt p) m -> p kt m", p=P), [P, KT, 3 * H], "r_gks")
            wdec = wp.tile([P, KT, H], bf16, name="wdec")
            round_into(wdec[:], wdec_d.rearrange("(kt p) m -> p kt m", p=P),
                       [P, KT, H], "r_wdec")
            wout = wp.tile([P, KT, NA], bf16, name="wout")
            round_into(wout[:], wout_d.rearrange("(kt p) m -> p kt m", p=P),
                       [P, KT, NA], "r_wout")

            benc = wp.tile([P, MT_H], f32, name="benc")
            nc.sync.dma_start(benc[:], benc_d.rearrange("(mt p) -> p mt", p=P))
            benc2 = wp.tile([P, MT_H], f32, name="benc2")
            nc.sync.dma_start(benc2[:], benc2_d.rearrange("(mt p) -> p mt", p=P))
            bdec = wp.tile([P, MT_H], f32, name="bdec")
            nc.sync.dma_start(bdec[:], bdec_d.rearrange("(mt p) -> p mt", p=P))
            bout = wp.tile([NA, 1], f32, name="bout")
            nc.sync.dma_start(bout[:], bout_d[:])
            cmb = wp.tile([P, C, MT_G], f32, name="cmb")
            nc.sync.dma_start(cmb[:], cmb_d.rearrange("c (mt p) -> p c mt", p=P))
            b1h = wp.tile([P, C, MT_H], f32, name="b1h")
            nc.sync.dma_start(b1h[:], b1h_d.rearrange("c (mt p) -> p c mt", p=P))
            e32 = wp.tile([64, CHUNK], bf16, name="e32")
            round_into(e32[:], e32_d[:], [64, CHUNK], "r_e32")

            # ---- main loop over super-chunks ----
            import contextlib
            rep_ctx = (tc.For_i(0, N_REPS, 1) if N_REPS > 1
                       else contextlib.nullcontext())
            with rep_ctx:
              for sc in range(N_SC):
                  h0 = scp.tile([P, MT_H, SC_ROWS], bf16, name="h0", tag="h0")
                  s1 = sp.tile([P, KT, SC_SAMPLES], bf16, name="s1", tag="seg")
                  s2 = sp.tile([P, KT, SC_SAMPLES], bf16, name="s2", tag="seg")
                  x1 = xp.tile([P, MT_H, SC_ROWS], bf16, name="x1", tag="x")
                  x2 = x1  # layer 2 overwrites layer-1 state chunk by chunk

                  # ---------- phase 1: obs -> enc -> enc2 -> h0, segsum s1 ----
                  for ch in range(N_CH):
                      r0 = sc * SC_ROWS + ch * CHUNK   # global row offset
                      co = ch * CHUNK                  # row offset within sc
                      obs_raw = tp.tile([D, CHUNK], f32, name="obs_raw")
                      nc.sync.dma_start(obs_raw[:], obs_d[:, r0:r0 + CHUNK])
                      obs_t = tp.tile([D, CHUNK], f32r, name="obs_t")
                      nc.vector.tensor_copy(obs_t[:], obs_raw[:])
                      # enc (K=64), then enc2 (K=256)
                      x1t = tp.tile([P, MT_H, CHUNK], f32r, name="x1t", tag="mlp_tmp")
                      for mt in range(MT_H):
                          pse = pm.tile([P, CHUNK], f32, name="pse", tag="mm")
                          nc.tensor.matmul(
                              pse[:], wenc[:, mt * P:(mt + 1) * P], obs_t[:],
                              start=True, stop=True)
                          nc.scalar.activation(
                              x1t[:, mt, :], pse[:], AF.Relu,
                              bias=benc[:, mt:mt + 1])
                      for mt in range(MT_H):
                          pse2 = pm.tile([P, CHUNK], f32, name="pse2", tag="mm")
                          for kt in range(KT):
                              nc.tensor.matmul(
                                  pse2[:],
                                  wenc2[:, kt, mt * P:(mt + 1) * P],
                                  x1t[:, kt, :],
                                  start=(kt == 0), stop=(kt == KT - 1))
                          nc.scalar.activation(
                              h0[:, mt, co:co + CHUNK], pse2[:], AF.Relu,
                              bias=benc2[:, mt:mt + 1])
                      # segsum of h0 chunk -> s1
                      with nc.allow_low_precision("f32r seg sums"):
                          nc.vector.tensor_reduce(
                              s1[:, :, ch * CH_S:(ch + 1) * CH_S],
                              h0[:, :, co:co + CHUNK].rearrange(
                                  "p mt (s a) -> p mt s a", a=A),
                              axis=mybir.AxisListType.X, op=ALU.add)

                  # ---------- GRU layers ----------
                  for li in range(C):
                      xin = h0 if li == 0 else x1
                      xout = x1 if li == 0 else x2
                      sseg = s1 if li == 0 else s2

                      # phase 2/4a: cm_h (h-gate slice, gate-major) for the
                      # hp broadcast add; constant bias folded in the evac
                      cm = sp.tile([P, MT_H, SC_SAMPLES], bf16, name="cm",
                                   tag="cm")
                      for mt in range(4, MT_G):
                          psc = pm.tile([P, SC_SAMPLES], f32, name="psc", tag="mm")
                          for kt in range(KT):
                              nc.tensor.matmul(
                                  psc[:],
                                  gks[:, li, kt, mt * P:(mt + 1) * P],
                                  sseg[:, kt, :],
                                  start=(kt == 0), stop=(kt == KT - 1))
                          nc.scalar.activation(
                              cm[:, mt - 4, :], psc[:], AF.Identity,
                              bias=cmb[:, li, mt:mt + 1])
                      # phase 2/4b: cmt = transposed cm for z,r gates
                      # (samples on partitions, 2 chunk-groups per tile);
                      # constants ride the sigmoid bias instead
                      cmts = []
                      for blk in range(N_CH // 2):
                          pct = pm.tile([64, 2 * H], f32, name="pct", tag="ct")
                          for g2 in range(2):
                              g = blk * 2 + g2
                              for kt in range(KT):
                                  nc.tensor.matmul(
                                      pct[g2 * 32:(g2 + 1) * 32, :],
                                      sseg[:, kt, g * CH_S:(g + 1) * CH_S],
                                      gks[:, li, kt, 0:2 * H],
                                      start=(kt == 0), stop=(kt == KT - 1),
                                      tile_position=(0, g2 * 32))
                          cmt = sp.tile([64, 2 * H], bf16, name="cmt", tag="cmt")
                          nc.scalar.copy(cmt[:], pct[:])
                          cmts.append(cmt)

                      # phase 3/5: per-chunk GRU pointwise
                      for ch in range(N_CH):
                          co = ch * CHUNK
                          ss = ch * CH_S
                          zt = tp.tile([P, MT_H, CHUNK], bf16, name="zt")
                          rt = tp.tile([P, MT_H, CHUNK], bf16, name="rt")
                          hh = tp.tile([P, MT_H, CHUNK], bf16, name="hh")
                          # gate psums: mh = x @ gru_rk
                          for mt in range(MT_G):
                              psg = pg.tile([P, CHUNK], f32, name="psg")
                              for kt in range(KT):
                                  nc.tensor.matmul(
                                      psg[:],
                                      grk[:, li, kt, mt * P:(mt + 1) * P],
                                      xin[:, kt, co:co + CHUNK],
                                      start=(kt == 0),
                                      stop=(kt == KT - 1 and mt >= 4))
                              if mt < 4:
                                  # z,r: cm broadcast via expander matmul into
                                  # psum, then sigmoid w/ folded const bias
                                  po = 32 * (ch % 2)
                                  nc.tensor.matmul(
                                      psg[:],
                                      cmts[ch // 2][po:po + 32,
                                                    mt * P:(mt + 1) * P],
                                      e32[po:po + 32, :],
                                      start=False, stop=True)
                                  dst = zt if mt < 2 else rt
                                  nc.scalar.activation(
                                      dst[:, mt % 2, :], psg[:], AF.Sigmoid,
                                      bias=cmb[:, li, mt:mt + 1])
                              else:
                                  # h gate: rrh = r * (mh_h + b1h) per m-tile
                                  # (per-partition scalar differs per m-tile)
                                  mtl = mt - 4
                                  if mtl == 0:
                                      rrh = tp.tile([P, MT_H, CHUNK], bf16,
                                                    name="rrh")
                                  nc.vector.scalar_tensor_tensor(
                                      rrh[:, mtl, :], psg[:],
                                      b1h[:, li, mtl:mtl + 1],
                                      rt[:, mtl, :],
                                      op0=ALU.add, op1=ALU.mult)
                          # merged across both h m-tiles: one broadcast add
                          # + one tanh (fewer, wider DVE/ACT ops)
                          cmv2 = cm[:, :, ss:ss + CH_S].unsqueeze(
                              3).broadcast_to([P, MT_H, CH_S, A])
                          hp = tp.tile([P, MT_H, CHUNK], bf16, name="hp")
                          nc.vector.tensor_tensor(
                              hp.rearrange("p mt (s a) -> p mt s a", a=A),
                              rrh.rearrange("p mt (s a) -> p mt s a", a=A),
                              cmv2, op=ALU.add)
                          nc.scalar.activation(hh[:], hp[:], AF.Tanh)
                          # combine: x_new = hh + z*(x - hh) + h0
                          dtl = tp.tile([P, MT_H, CHUNK], bf16, name="dtl")
                          nc.vector.tensor_tensor(
                              dtl[:], xin[:, :, co:co + CHUNK], hh[:],
                              op=ALU.subtract)
                          nc.vector.tensor_tensor(dtl[:], zt[:], dtl[:],
                                                  op=ALU.mult)
                          nc.vector.tensor_tensor(dtl[:], dtl[:], hh[:],
                                                  op=ALU.add)
                          nc.vector.tensor_tensor(
                              xout[:, :, co:co + CHUNK], dtl[:],
                              h0[:, :, co:co + CHUNK], op=ALU.add)
                          if li == 0:
                              with nc.allow_low_precision("f32r seg sums"):
                                  nc.vector.tensor_reduce(
                                      s2[:, :, ss:ss + CH_S],
                                      xout[:, :, co:co + CHUNK].rearrange(
                                          "p mt (s a) -> p mt s a", a=A),
                                      axis=mybir.AxisListType.X, op=ALU.add)
                          else:
                              # ---- dec + out fused into phase 5 ----
                              dd = tp.tile([P, MT_H, CHUNK], bf16, name="dd")
                              for mt in range(MT_H):
                                  psd = pm.tile([P, CHUNK], f32, name="psd", tag="mm")
                                  for kt in range(KT):
                                      nc.tensor.matmul(
                                          psd[:],
                                          wdec[:, kt, mt * P:(mt + 1) * P],
                                          xout[:, kt, co:co + CHUNK],
                                          start=(kt == 0), stop=(kt == KT - 1))
                                  nc.scalar.activation(
                                      dd[:, mt, :], psd[:], AF.Relu,
                                      bias=bdec[:, mt:mt + 1])
                              pso = pm.tile([NA, CHUNK], f32, name="pso", tag="mm")
                              for kt in range(KT):
                                  nc.tensor.matmul(
                                      pso[:], wout[:, kt, :], dd[:, kt, :],
                                      start=(kt == 0), stop=(kt == KT - 1))
                              ot = tp.tile([NA, CHUNK], f32, name="ot")
                              nc.scalar.add(ot[:], pso[:], bout[:])
                              nc.sync.dma_start(
                                  out_d[:, sc * SC_ROWS + co:
                                        sc * SC_ROWS + co + CHUNK], ot[:])

    nc.compile()
    return nc


def _host_prep(inputs):
    """Host-side preprocessing of weights/constants (tiny, O(H^2))."""
    g = lambda k: np.asarray(inputs[k], np.float32)
    obs = g("obs")
    mask = g("mask")            # (1, A, 1)
    ou_s0, ou_s1 = g("ou_s0"), g("ou_s1")   # (C,1,A,H)
    ou_s2, ou_s3 = g("ou_s2"), g("ou_s3")   # (C,1,1,H)
    gru_k, gru_b = g("gru_k"), g("gru_b")

    m = mask[0, :, :]                        # (A, 1)
    cmb = np.zeros((C, 3 * H), np.float32)
    b1h = np.zeros((C, H), np.float32)
    gks = np.zeros((C, H, 3 * H), np.float32)
    for i in range(C):
        send = (m * ou_s1[i, 0] * ou_s0[i, 0]).sum(0) / A      # (H,)
        recv = (m.mean(0) * ou_s3[i, 0, 0] * ou_s2[i, 0, 0])   # (H,)
        const = (send + recv).astype(np.float64) @ gru_k[i].astype(np.float64)
        cmbi = const + gru_b[i, 0].astype(np.float64)
        cmbi[:2 * H] += gru_b[i, 1, :2 * H].astype(np.float64)
        cmb[i] = cmbi.astype(np.float32)
        b1h[i] = gru_b[i, 1, 2 * H:]
        gks[i] = gru_k[i] / A

    shared = {
        "wenc": g("W_enc"), "benc": g("b_enc"),
        "wenc2": g("W_enc2"), "benc2": g("b_enc2"),
        "grk": g("gru_rk"), "gks": gks, "cmb": cmb, "b1h": b1h,
        "wdec": g("W_dec"), "bdec": g("b_dec"),
        "wout": g("W_out"), "bout": g("b_out").reshape(NA, 1),
    }
    # per-core transposed obs (layout choice; full data still streamed on-device)
    e32 = np.zeros((64, CHUNK), np.float32)
    for g2 in range(2):
        for s in range(CH_S):
            e32[g2 * 32 + s, s * A:(s + 1) * A] = 1.0
    shared["e32"] = e32
    obs_r = np.ascontiguousarray(
        obs.reshape(NCORES, R_LOC, D).transpose(0, 2, 1))
    in_maps = [dict(shared, obs=obs_r[k]) for k in range(NCORES)]
    return in_maps


def kernel(**inputs):
    if "nc" not in _CACHE:
        _CACHE["nc"] = _build()
    nc = _CACHE["nc"]
    in_maps = _host_prep(inputs)
    res = run_bass_kernel_spmd(nc, in_maps, core_ids=list(range(NCORES)))
    outs = np.stack([res.results[k]["out_t"] for k in range(NCORES)])
    # (8, 5, 16384) -> (8192, 16, 5)
    out = outs.reshape(NCORES, NA, B_LOC, A).transpose(0, 2, 3, 1)
    return np.ascontiguousarray(out.reshape(B, A, NA)).astype(np.float32)



# revision 2
# speedup vs baseline: 1.4063x; 1.4063x over previous
"""TRN2 Bass kernel for nn_CNActorController (GRU comm-net actor), v2.

Strategy (self-contained, shapes hardcoded):
- Pure data parallelism: batch dim sharded 8 ways (1024 samples / core).
- Feature-major on-chip layout: activations [feature partitions, rows],
  rows sample-major (b*16+a).
- Agent-mean communication collapses to per-sample segsums + small
  matmuls; all OU-noise terms and gate biases fold into per-gate
  constants added via ones-row matmuls (so sigmoid evacs need no bias
  and merge across m-tiles).
- GRU gate matmuls run in fp8(e4m3) DoubleRow mode: K=256 contracted in
  one PE pass at 2 rows/cycle. Everything else bf16.
- Elementwise work is spread across ACT / DVE / Pool(GpSimd) engines;
  assignment below was tuned with the instruction-cost timeline sim.
"""
import os
import numpy as np
import ml_dtypes

import jax
try:
    _cache_dir = os.path.expanduser("~/.cache/jax_kernel_cache")
    os.makedirs(_cache_dir, exist_ok=True)
    jax.config.update("jax_compilation_cache_dir", _cache_dir)
    jax.config.update("jax_persistent_cache_min_entry_size_bytes", -1)
    jax.config.update("jax_persistent_cache_min_compile_time_secs", 2)
except Exception:
    pass

import concourse.bass as bass
import concourse.mybir as mybir
import concourse.tile as tile
from concourse import bacc
from concourse.bass_utils import run_bass_kernel_spmd

# problem dims (hardcoded per spec)
B, A, D, H, C, NA = 8192, 16, 64, 256, 2, 5
NCORES = 8
B_LOC = B // NCORES          # 1024 samples per core
R_LOC = B_LOC * A            # 16384 rows per core
P = 128

SC_SAMPLES = 256             # samples per super-chunk
N_SC = B_LOC // SC_SAMPLES   # 4 super-chunks
CHUNK = 512                  # rows per chunk
CH_S = CHUNK // A            # 32 samples per chunk
N_CH = SC_SAMPLES * A // CHUNK  # 8 chunks per super-chunk
SC_ROWS = SC_SAMPLES * A     # 4096 rows per super-chunk

MT_G = 6                     # gate m-tiles (3H/128)
MT_H = 2                     # state m-tiles (H/128)
KT = 2                       # k-tiles for H contraction

f32 = mybir.dt.float32
bf16 = mybir.dt.bfloat16
fp8 = mybir.dt.float8e4
AF = mybir.ActivationFunctionType
ALU = mybir.AluOpType
DR = mybir.MatmulPerfMode.DoubleRow

_CACHE = {}
N_REPS = 1

# --- engine assignment flags (tuned on hardware) ---
# HW findings: Pool(GpSimd) tensor ops are catastrophically slow (software
# Q7 path) — keep it idle. DVE tensor_copy with fp8 output is also very
# slow on HW; the state casts run on ACT instead.
GATES_FP8 = True      # fp8 DoubleRow for x@gru_rk (both layers)
S1_ENGINE = "dve"     # segsum of h0 (free-axis reduce is DVE-only)
S2_ENGINE = "dve"     # segsum of x1
HP_ENGINE = "dve"     # hp = rrh + cm broadcast
U_ENGINE = "dve"      # u = hh + h0
DEC_LAG = 3           # chunks of deferral for dec/out in layer 2
B_LAG = 1             # stage-B pipeline depth behind stage A
CAST_ENGINE = "act"   # bf16 -> fp8 state cast (layer 1)
CAST1_ENGINE = "act"  # bf16 -> fp8 h0 cast (phase 1)
OT_EVAC = "dve"       # out psum + bout evac
CMT_EVAC = "act"      # pct psum copy evac


def _build():
    """Build the per-core Bass module (same program for all 8 cores)."""
    nc = bacc.Bacc("TRN2", target_bir_lowering=False, debug=False,
                   num_devices=NCORES)

    # ---- DRAM I/O (all pre-cast on host to final on-chip dtype) ----
    obs_d = nc.dram_tensor("obs", [D, R_LOC], bf16, kind="ExternalInput")
    wenc_d = nc.dram_tensor("wenc", [D + 1, H], bf16, kind="ExternalInput")
    wenc2_d = nc.dram_tensor("wenc2", [P, KT, H], bf16, kind="ExternalInput")
    benc2_d = nc.dram_tensor("benc2", [H], bf16, kind="ExternalInput")
    grk8_d = nc.dram_tensor("grk8", [P, C, KT, 3 * H], fp8,
                            kind="ExternalInput")
    grkb_d = nc.dram_tensor("grkb", [P, C, KT, 3 * H], bf16,
                            kind="ExternalInput")
    gks_d = nc.dram_tensor("gks", [P, C, KT, 3 * H], bf16,
                           kind="ExternalInput")
    cmbzr_d = nc.dram_tensor("cmbzr", [C, 2 * H], bf16, kind="ExternalInput")
    cmbh_d = nc.dram_tensor("cmbh", [C, H], f32, kind="ExternalInput")
    b1h_d = nc.dram_tensor("b1h", [C, H], f32, kind="ExternalInput")
    wdec_d = nc.dram_tensor("wdec", [P, KT, H], bf16, kind="ExternalInput")
    bdec_d = nc.dram_tensor("bdec", [H], bf16, kind="ExternalInput")
    wout_d = nc.dram_tensor("wout", [P, KT, NA], bf16, kind="ExternalInput")
    bout_d = nc.dram_tensor("bout", [NA, 1], f32, kind="ExternalInput")
    e32_d = nc.dram_tensor("e32", [P, CHUNK], bf16, kind="ExternalInput")
    out_d = nc.dram_tensor("out_t", [NA, R_LOC], f32, kind="ExternalOutput")

    with tile.TileContext(nc) as tc:
        with (
            tc.tile_pool(name="wp", bufs=1) as wp,          # weights, consts
            tc.tile_pool(name="ob", bufs=3) as obp,         # obs staging
            tc.tile_pool(name="sc", bufs=2) as scp,         # per-sc h0 (+fp8)
            tc.tile_pool(name="xp", bufs=2) as xp,          # x state buffer
            tc.tile_pool(name="sp", bufs=2) as sp,          # segsums + cm
            tc.tile_pool(name="tp", bufs=3) as tp,          # chunk transients
            tc.tile_pool(name="pz", bufs=2, space="PSUM") as pz,  # [P,2,512]
            tc.tile_pool(name="ph", bufs=3, space="PSUM") as ph,  # [P,512]
            tc.tile_pool(name="pt", bufs=1, space="PSUM") as pt,  # pct
        ):
            # ---- load weights (direct DMA, host pre-cast) ----
            wenc = wp.tile([D + 1, H], bf16, name="wenc")
            nc.sync.dma_start(wenc[:], wenc_d[:])
            wenc2 = wp.tile([P, KT, H], bf16, name="wenc2")
            nc.sync.dma_start(wenc2[:], wenc2_d[:])
            if GATES_FP8:
                grk8 = wp.tile([P, C, KT, 3 * H], fp8, name="grk8")
                nc.sync.dma_start(grk8[:], grk8_d[:])
            else:
                grkb = wp.tile([P, C, KT, 3 * H], bf16, name="grkb")
                nc.sync.dma_start(grkb[:], grkb_d[:])
            gks = wp.tile([P, C, KT, 3 * H], bf16, name="gks")
            nc.sync.dma_start(gks[:], gks_d[:])
            wdec = wp.tile([P, KT, H], bf16, name="wdec")
            nc.sync.dma_start(wdec[:], wdec_d[:])
            wout = wp.tile([P, KT, NA], bf16, name="wout")
            nc.sync.dma_start(wout[:], wout_d[:])

            benc2r = wp.tile([1, H], bf16, name="benc2r")
            nc.sync.dma_start(benc2r[:], benc2_d.rearrange("(o m) -> o m", o=1))
            bdecr = wp.tile([1, H], bf16, name="bdecr")
            nc.sync.dma_start(bdecr[:], bdec_d.rearrange("(o m) -> o m", o=1))
            bout = wp.tile([NA, 1], f32, name="bout")
            nc.sync.dma_start(bout[:], bout_d[:])
            cmbzr = wp.tile([1, C, 2 * H], bf16, name="cmbzr")
            nc.sync.dma_start(cmbzr[:], cmbzr_d.rearrange("(o c) m -> o c m", o=1))
            cmbh = wp.tile([P, C, MT_H], f32, name="cmbh")
            nc.sync.dma_start(cmbh[:], cmbh_d.rearrange("c (mt p) -> p c mt", p=P))
            b1h = wp.tile([P, C, MT_H], f32, name="b1h")
            nc.sync.dma_start(b1h[:], b1h_d.rearrange("c (mt p) -> p c mt", p=P))
            e32 = wp.tile([P, CHUNK], bf16, name="e32")
            nc.sync.dma_start(e32[:], e32_d[:])
            ones = wp.tile([1, CHUNK], bf16, name="ones")
            nc.vector.memset(ones[:], 1.0)

            # obs staging tiles with a constant-1 bias row (row D)
            obs_tiles = []
            for i in range(3):
                t = wp.tile([D + 1, CHUNK], bf16, name=f"obst{i}")
                nc.vector.memset(t[D:D + 1, :], 1.0)
                obs_tiles.append(t)

            def seg_reduce(eng, dst_ap, src_ap):
                with nc.allow_low_precision("bf16 seg sums"):
                    eng.tensor_reduce(
                        dst_ap,
                        src_ap.rearrange("p mt (s a) -> p mt s a", a=A),
                        axis=mybir.AxisListType.X, op=ALU.add)

            eng_of = {"dve": nc.vector, "pool": nc.gpsimd, "act": nc.scalar}

            def cast_copy(eng_name, dst_ap, src_ap):
                if eng_name == "act":
                    nc.scalar.copy(dst_ap, src_ap)
                else:
                    eng_of[eng_name].tensor_copy(dst_ap, src_ap)

            # ---- main loop over super-chunks ----
            import contextlib
            rep_ctx = (tc.For_i(0, N_REPS, 1) if N_REPS > 1
                       else contextlib.nullcontext())
            # sc body as a function: per-sc tiles live in this frame, so
            # closures carried into the next sc keep the right bindings
            def do_sc(sc, carry):
                pend_b = []
                h0 = scp.tile([P, MT_H, SC_ROWS], bf16, name="h0", tag="h0")
                h08 = scp.tile([P, MT_H, SC_ROWS], fp8, name="h08", tag="h08")
                s1 = sp.tile([P, KT, SC_SAMPLES], bf16, name="s1", tag="seg",
                             bufs=4)
                s2 = sp.tile([P, KT, SC_SAMPLES], bf16, name="s2", tag="seg",
                             bufs=4)
                x1 = xp.tile([P, MT_H, SC_ROWS], bf16, name="x1", tag="x")
                x18 = xp.tile([P, MT_H, SC_ROWS], fp8, name="x18", tag="x8")
                x2 = x1  # layer 2 overwrites layer-1 state chunk by chunk

                # per-layer comm tiles, built incrementally in 64-sample
                # blocks as segsums land (keeps the in-order PE stream from
                # stalling at phase boundaries)
                cmL = {}
                cmtsL = {0: [], 1: []}

                def build_cm_blk(li, b):
                    sseg = s1 if li == 0 else s2
                    c0 = b * 64
                    if li not in cmL:
                        cmL[li] = sp.tile([P, MT_H, SC_SAMPLES], bf16,
                                          name="cm", tag="cm", bufs=4)
                    cm = cmL[li]
                    # cm_h (feature-major) for this sample block
                    for mt in range(MT_H):
                        pscb = ph.tile([P, 64], f32, name="pscb", tag="h")
                        for kt in range(KT):
                            nc.tensor.matmul(
                                pscb[:],
                                gks[:, li, kt, (4 + mt) * P:(5 + mt) * P],
                                sseg[:, kt, c0:c0 + 64],
                                start=(kt == 0), stop=(kt == KT - 1))
                        nc.scalar.activation(
                            cm[:, mt, c0:c0 + 64], pscb[:], AF.Identity,
                            bias=cmbh[:, li, mt:mt + 1])
                    # cmt: transposed cm for z,r gates (samples on
                    # partitions); zr constants ride a ones-row matmul so
                    # the sigmoid needs no bias
                    pct = pt.tile([64, 2 * H], f32, name="pct", tag="ct")
                    for st in range(2):
                        s0 = c0 + st * 32
                        for kt in range(KT):
                            nc.tensor.matmul(
                                pct[st * 32:(st + 1) * 32, :],
                                sseg[:, kt, s0:s0 + 32],
                                gks[:, li, kt, 0:2 * H],
                                start=(kt == 0), stop=False,
                                tile_position=(0, st * 32))
                    nc.tensor.matmul(
                        pct[:], ones[:, 0:64], cmbzr[:, li, :],
                        start=False, stop=True, skip_group_check=True)
                    cmt = sp.tile([64, 2 * H], bf16, name="cmt",
                                  tag="cmt", bufs=12)
                    if CMT_EVAC == "act":
                        nc.scalar.copy(cmt[:], pct[:])
                    else:
                        eng_of[CMT_EVAC].tensor_copy(cmt[:], pct[:])
                    cmtsL[li].append(cmt)

                # dec+out for one finished chunk (deferred so the PE
                # stream never blocks on the pointwise chain's xout)
                def decout(ch):
                    co = ch * CHUNK
                    psd = pz.tile([P, MT_H, CHUNK], f32, name="psd",
                                  tag="z")
                    for mt in range(MT_H):
                        for kt in range(KT):
                            nc.tensor.matmul(
                                psd[:, mt, :],
                                wdec[:, kt, mt * P:(mt + 1) * P],
                                x2[:, kt, co:co + CHUNK],
                                start=(kt == 0), stop=False)
                        nc.tensor.matmul(
                            psd[:, mt, :], bdecr[:, mt * P:(mt + 1) * P],
                            ones[:], start=False, stop=True,
                            skip_group_check=True)
                    dd = tp.tile([P, MT_H, CHUNK], bf16, name="dd")
                    nc.scalar.activation(dd[:], psd[:], AF.Relu)
                    pso = ph.tile([NA, CHUNK], f32, name="pso", tag="h")
                    for kt in range(KT):
                        nc.tensor.matmul(
                            pso[:], wout[:, kt, :], dd[:, kt, :],
                            start=(kt == 0), stop=(kt == KT - 1))
                    ot = tp.tile([NA, CHUNK], f32, name="ot")
                    if OT_EVAC == "act":
                        nc.scalar.add(ot[:], pso[:], bout[:])
                    else:
                        eng_of[OT_EVAC].tensor_scalar(
                            ot[:], pso[:], bout[:], None, op0=ALU.add)
                    nc.sync.dma_start(
                        out_d[:, sc * SC_ROWS + co:
                              sc * SC_ROWS + co + CHUNK], ot[:])

                # stage A: all PE matmuls + sigmoids + rrh for one chunk
                def gate_mm(dst_ap, li, mt_abs, co, xin, xin8, last):
                    m0 = mt_abs * P
                    if GATES_FP8:
                        nc.tensor.matmul(
                            dst_ap, grk8[:, li, :, m0:m0 + P],
                            xin8[:, :, co:co + CHUNK],
                            start=True, stop=last and False, perf_mode=DR,
                            skip_group_check=True)
                    else:
                        for kt in range(KT):
                            nc.tensor.matmul(
                                dst_ap, grkb[:, li, kt, m0:m0 + P],
                                xin[:, kt, co:co + CHUNK],
                                start=(kt == 0), stop=False,
                                skip_group_check=True)

                def stageA(li, ch):
                    xin = h0 if li == 0 else x1
                    xin8 = (h08 if li == 0 else x18) if GATES_FP8 else None
                    co = ch * CHUNK
                    cmt = cmtsL[li][ch // 2]
                    po = (ch % 2) * 32
                    zt = tp.tile([P, MT_H, CHUNK], bf16, name="zt")
                    rt = tp.tile([P, MT_H, CHUNK], bf16, name="rt")
                    for gi, dst in ((1, rt), (0, zt)):   # r first: rrh needs it
                        psg = pz.tile([P, MT_H, CHUNK], f32, name="psg",
                                      tag="z")
                        for mt in range(MT_H):
                            m0 = (gi * MT_H + mt) * P
                            gate_mm(psg[:, mt, :], li, gi * MT_H + mt, co,
                                    xin, xin8, last=False)
                            nc.tensor.matmul(
                                psg[:, mt, :],
                                cmt[po:po + 32, m0:m0 + P],
                                e32[po:po + 32, :], start=False,
                                stop=True, skip_group_check=True)
                        nc.scalar.activation(dst[:], psg[:], AF.Sigmoid)
                    # h gate: matmul per m-tile, rrh = (mh+b1h)*r
                    rrh = tp.tile([P, MT_H, CHUNK], bf16, name="rrh")
                    for mt in range(MT_H):
                        psh = ph.tile([P, CHUNK], f32, name="psh", tag="h")
                        m0 = (4 + mt) * P
                        if GATES_FP8:
                            nc.tensor.matmul(
                                psh[:], grk8[:, li, :, m0:m0 + P],
                                xin8[:, :, co:co + CHUNK],
                                start=True, stop=True, perf_mode=DR)
                        else:
                            for kt in range(KT):
                                nc.tensor.matmul(
                                    psh[:], grkb[:, li, kt, m0:m0 + P],
                                    xin[:, kt, co:co + CHUNK],
                                    start=(kt == 0), stop=(kt == KT - 1))
                        nc.vector.scalar_tensor_tensor(
                            rrh[:, mt, :], psh[:],
                            b1h[:, li, mt:mt + 1],
                            rt[:, mt, :],
                            op0=ALU.add, op1=ALU.mult)
                    return (li, ch, zt, rrh)

                # stage B: pointwise tail (lags one chunk behind stage A)
                def stageB(li, ch, zt, rrh):
                    xin = h0 if li == 0 else x1
                    xout = x1 if li == 0 else x2
                    co = ch * CHUNK
                    ss = ch * CH_S
                    cm = cmL[li]
                    cmv2 = cm[:, :, ss:ss + CH_S].unsqueeze(
                        3).broadcast_to([P, MT_H, CH_S, A])
                    hp = tp.tile([P, MT_H, CHUNK], bf16, name="hp")
                    eng_of[HP_ENGINE].tensor_tensor(
                        hp.rearrange("p mt (s a) -> p mt s a", a=A),
                        rrh.rearrange("p mt (s a) -> p mt s a", a=A),
                        cmv2, op=ALU.add)
                    hh = tp.tile([P, MT_H, CHUNK], bf16, name="hh")
                    nc.scalar.activation(hh[:], hp[:], AF.Tanh)
                    # combine: x_new = z*(x - hh) + (hh + h0)
                    vv = tp.tile([P, MT_H, CHUNK], bf16, name="vv")
                    nc.vector.tensor_tensor(
                        vv[:], xin[:, :, co:co + CHUNK], hh[:],
                        op=ALU.subtract)
                    uu = tp.tile([P, MT_H, CHUNK], bf16, name="uu")
                    eng_of[U_ENGINE].tensor_tensor(
                        uu[:], hh[:], h0[:, :, co:co + CHUNK], op=ALU.add)
                    nc.vector.tensor_tensor(vv[:], zt[:], vv[:], op=ALU.mult)
                    nc.vector.tensor_tensor(
                        xout[:, :, co:co + CHUNK], vv[:], uu[:], op=ALU.add)
                    if li == 0:
                        if GATES_FP8:
                            cast_copy(CAST_ENGINE,
                                      x18[:, :, co:co + CHUNK],
                                      xout[:, :, co:co + CHUNK])
                        seg_reduce(eng_of[S2_ENGINE],
                                   s2[:, :, ss:ss + CH_S],
                                   xout[:, :, co:co + CHUNK])

                def pop_b(force=True):
                    if pend_b and (force or len(pend_b) > B_LAG):
                        args = pend_b.pop(0)
                        stageB(*args)

                # ---------- phase 1: obs -> enc -> enc2 -> h0, segsum s1 ----
                for ch in range(N_CH):
                    r0 = sc * SC_ROWS + ch * CHUNK   # global row offset
                    co = ch * CHUNK                  # row offset within sc
                    obs_t = obs_tiles[(sc * N_CH + ch) % 3]
                    nc.sync.dma_start(obs_t[0:D, :], obs_d[:, r0:r0 + CHUNK])
                    # enc1 (K=65 incl. bias row) -> merged relu evac
                    pse = pz.tile([P, MT_H, CHUNK], f32, name="pse", tag="z")
                    for mt in range(MT_H):
                        nc.tensor.matmul(
                            pse[:, mt, :], wenc[:, mt * P:(mt + 1) * P],
                            obs_t[:], start=True, stop=True)
                    x1t = tp.tile([P, MT_H, CHUNK], bf16, name="x1t",
                                  tag="mlp_tmp")
                    nc.scalar.activation(x1t[:], pse[:], AF.Relu)
                    # enc2 (K=256, bias via ones-row) -> merged relu evac
                    pse2 = pz.tile([P, MT_H, CHUNK], f32, name="pse2", tag="z")
                    for mt in range(MT_H):
                        for kt in range(KT):
                            nc.tensor.matmul(
                                pse2[:, mt, :],
                                wenc2[:, kt, mt * P:(mt + 1) * P],
                                x1t[:, kt, :],
                                start=(kt == 0), stop=False)
                        nc.tensor.matmul(
                            pse2[:, mt, :], benc2r[:, mt * P:(mt + 1) * P],
                            ones[:], start=False, stop=True,
                            skip_group_check=True)
                    nc.scalar.activation(h0[:, :, co:co + CHUNK], pse2[:],
                                         AF.Relu)
                    if GATES_FP8:
                        cast_copy(CAST1_ENGINE,
                                  h08[:, :, co:co + CHUNK],
                                  h0[:, :, co:co + CHUNK])
                    seg_reduce(eng_of[S1_ENGINE],
                               s1[:, :, ch * CH_S:(ch + 1) * CH_S],
                               h0[:, :, co:co + CHUNK])
                    # flush work carried over from the previous super-chunk
                    # (one item per chunk; stage-Bs first, then decouts,
                    # so each carried chain drains before its tail is hit)
                    if ch < len(carry):
                        carry[ch]()
                    if ch in (3, 5, 7):
                        build_cm_blk(0, (ch - 3) // 2)
                carry = []

                # ---------- GRU layers (stage-A/stage-B pipelined) ----------
                for li in range(C):
                    for ch in range(N_CH):
                        a = stageA(li, ch)
                        pend_b.append(a)
                        pop_b(force=False)
                        if li == 0:
                            if ch == 1:
                                build_cm_blk(0, 3)
                            elif ch in (5, 7):
                                build_cm_blk(1, (ch - 5) // 2)
                        else:
                            if ch in (1, 3):
                                build_cm_blk(1, 2 + (ch - 1) // 2)
                            if ch >= DEC_LAG:
                                decout(ch - DEC_LAG)

                # defer the superchunk tail into the next sc's phase 1
                out = []
                while pend_b:
                    out.append((lambda f=stageB, a=pend_b.pop(0): f(*a)))
                for c in range(N_CH - DEC_LAG, N_CH):
                    out.append((lambda f=decout, c=c: f(c)))
                return out

            with rep_ctx:
                carry = []
                for sc in range(N_SC):
                    carry = do_sc(sc, carry)
                for fn_ in carry:
                    fn_()

    nc.compile()
    return nc


def _host_prep(inputs):
    """Host-side preprocessing of weights/constants (tiny, O(H^2))."""
    g = lambda k: np.asarray(inputs[k], np.float32)
    obs = g("obs")
    mask = g("mask")            # (1, A, 1)
    ou_s0, ou_s1 = g("ou_s0"), g("ou_s1")   # (C,1,A,H)
    ou_s2, ou_s3 = g("ou_s2"), g("ou_s3")   # (C,1,1,H)
    gru_k, gru_b = g("gru_k"), g("gru_b")

    m = mask[0, :, :]                        # (A, 1)
    cmb = np.zeros((C, 3 * H), np.float32)
    b1h = np.zeros((C, H), np.float32)
    gks = np.zeros((C, H, 3 * H), np.float32)
    for i in range(C):
        send = (m * ou_s1[i, 0] * ou_s0[i, 0]).sum(0) / A      # (H,)
        recv = (m.mean(0) * ou_s3[i, 0, 0] * ou_s2[i, 0, 0])   # (H,)
        const = (send + recv).astype(np.float64) @ gru_k[i].astype(np.float64)
        cmbi = const + gru_b[i, 0].astype(np.float64)
        cmbi[:2 * H] += gru_b[i, 1, :2 * H].astype(np.float64)
        cmb[i] = cmbi.astype(np.float32)
        b1h[i] = gru_b[i, 1, 2 * H:]
        gks[i] = gru_k[i] / A

    bf = ml_dtypes.bfloat16
    f8 = ml_dtypes.float8_e4m3

    def kt_split(w):  # [K, M] -> [P, KT, M]
        return np.ascontiguousarray(
            w.reshape(KT, P, -1).transpose(1, 0, 2))

    wenc_ext = np.concatenate(
        [g("W_enc"), g("b_enc").reshape(1, H)], axis=0)   # (65, 256)
    grk_t = np.stack([kt_split(g("gru_rk")[i]) for i in range(C)], axis=1)
    gks_t = np.stack([kt_split(gks[i]) for i in range(C)], axis=1)

    e32 = np.zeros((P, CHUNK), np.float32)
    for p in range(P):
        s = p % CH_S
        e32[p, s * A:(s + 1) * A] = 1.0

    shared = {
        "wenc": wenc_ext.astype(bf),
        "wenc2": kt_split(g("W_enc2")).astype(bf),
        "benc2": g("b_enc2").astype(bf),
        "grk8": grk_t.astype(f8),
        "grkb": grk_t.astype(bf),
        "gks": gks_t.astype(bf),
        "cmbzr": cmb[:, :2 * H].astype(bf),
        "cmbh": cmb[:, 2 * H:].copy(),
        "b1h": b1h,
        "wdec": kt_split(g("W_dec")).astype(bf),
        "bdec": g("b_dec").astype(bf),
        "wout": kt_split(g("W_out")).astype(bf),
        "bout": g("b_out").reshape(NA, 1),
        "e32": e32.astype(bf),
    }
    # per-core transposed obs (layout choice; full data still streamed)
    obs_r = np.ascontiguousarray(
        obs.reshape(NCORES, R_LOC, D).transpose(0, 2, 1)).astype(bf)
    in_maps = [dict(shared, obs=obs_r[k]) for k in range(NCORES)]
    return in_maps


def kernel(**inputs):
    if "nc" not in _CACHE:
        _CACHE["nc"] = _build()
    nc = _CACHE["nc"]
    in_maps = _host_prep(inputs)
    res = run_bass_kernel_spmd(nc, in_maps, core_ids=list(range(NCORES)))
    outs = np.stack([res.results[k]["out_t"] for k in range(NCORES)])
    # (8, 5, 16384) -> (8192, 16, 5)
    out = outs.reshape(NCORES, NA, B_LOC, A).transpose(0, 2, 3, 1)
    return np.ascontiguousarray(out.reshape(B, A, NA)).astype(np.float32)
